# revision 7
# baseline (speedup 1.0000x reference)
"""Trainium2 Bass kernel for the CP-decomposed 2-layer CNN + classifier.

Network (per image):
  x (3,32,32) -> CP conv1 -> h1 (32,30,30) -> CP conv2 -> h2 (32,28,28)
  -> flatten -> W_cls -> log_softmax  -> (10,)

The CP (rank-16) structure makes every layer separable. We fuse:
  MM1  = channel-contract (C3->R16) + vertical 3-tap conv of layer 1
         (taps folded into K via 3 shifted copies of x: K = 3*8img*3c = 72)
  MM2  = horizontal conv of layer 1 + (layer-1 expand @ layer-2 contract)
         as a single 16->16 pointwise map, 3 w-shifted matmuls PSUM-accumulated
  CLS  = layer-2 vertical+horizontal convs + layer-2 expand + classifier,
         all folded into one (10 x 16*30*30) weight matrix precomputed on
         host, applied on transposed (feature-major) activations.

Data-parallel over batch: 512 images -> 8 cores x 64 images.
Per core, images are processed in 8 groups of 8 (partition dim = 8 img x 16 rank).
"""

import sys

sys.path.insert(0, "/opt/trn_rl_repo")

import numpy as np
import ml_dtypes

import dataclasses

import concourse.bass as bass
import concourse.bacc as bacc
import concourse.mybir as mybir
import concourse.tile as tile
from concourse.bass_utils import run_bass_kernel_spmd

F32 = mybir.dt.float32
F32R = mybir.dt.float32r
BF16 = mybir.dt.bfloat16

N_CORES = 8
B = 512
B_LOC = B // N_CORES  # 64 images per core
G = 8                 # images per group
NG = B_LOC // G       # 8 groups
R = 16                # CP rank
NC = 10               # classes

# grids
H1, W1 = 30, 32       # t1 grid (after vertical conv of layer 1)
H2, W2 = 30, 30       # z2 grid (after horizontal conv + channel mix)
PIX2 = H2 * W2        # 900
NCHUNK = 8            # transpose chunks of 128 pix (last one overlaps, zero-masked)

_CACHE = {}


def _build_nc():
    nc = bacc.Bacc()
    x_d = nc.dram_tensor("x", [B_LOC * 3, 1024], BF16, kind="ExternalInput")
    w1_d = nc.dram_tensor("w1", [72, 128], BF16, kind="ExternalInput")
    w2_d = nc.dram_tensor("w2", [128, 3 * 128], BF16, kind="ExternalInput")
    wc_d = nc.dram_tensor("wc", [128, NCHUNK * R * NC], BF16, kind="ExternalInput")
    bc_d = nc.dram_tensor("bc", [B_LOC, NC], F32, kind="ExternalInput")
    out_d = nc.dram_tensor("out", [B_LOC, NC], F32, kind="ExternalOutput")

    with tile.TileContext(nc) as tc:
        with (
            tc.tile_pool(name="wpool", bufs=1) as wp,
            tc.tile_pool(name="xp", bufs=3) as xp,
            tc.tile_pool(name="t1p", bufs=2) as t1p,
            tc.tile_pool(name="z2p", bufs=2) as z2p,
            tc.tile_pool(name="tallp", bufs=1) as tallp,
            tc.tile_pool(name="smx", bufs=1) as smx,
            tc.tile_pool(name="ps1", bufs=2, space="PSUM") as ps1,
            tc.tile_pool(name="ps2", bufs=2, space="PSUM") as ps2,
            tc.tile_pool(name="psl", bufs=1, space="PSUM") as psl,
        ):
            w1 = wp.tile([72, 128], BF16)
            nc.sync.dma_start(w1[:, :], w1_d[:, :])
            w2 = wp.tile([128, 3 * 128], BF16)
            nc.sync.dma_start(w2[:, :], w2_d[:, :])
            wc = wp.tile([128, NCHUNK * R * NC], BF16)
            nc.sync.dma_start(wc[:, :], wc_d[:, :])
            bc = wp.tile([B_LOC, NC], F32)
            nc.sync.dma_start(bc[:, :], bc_d[:, :])

            tall = [
                tallp.tile([128, 1024], BF16, name=f"tall{c}", tag=f"tall{c}")
                for c in range(NCHUNK)
            ]

            for g in range(NG):
                # --- load x for this group: 3 dx-shifted copies on partitions
                xg = xp.tile([72, 1024], BF16)
                src = x_d[24 * g : 24 * g + 24, 0:960]
                src3 = dataclasses.replace(src, ap=[[32, 3]] + list(src.ap))
                nc.sync.dma_start(xg[0:72, 0:960], src3)

                # --- MM1: channel contract + vertical conv 1 -> t1[(img,r), (h30, w32)]
                t1 = t1p.tile([128, H1 * W1], BF16)
                xv = xg[0:72, :].rearrange("p (h w) -> p h w", w=32)
                for half in range(2):
                    pt = ps1.tile([128, 480], F32)
                    nc.tensor.matmul(
                        pt[:, :],
                        w1[:, :],
                        xv[:, 15 * half : 15 * half + 15, :],
                        start=True,
                        stop=True,
                    )
                    dst = t1[:, 480 * half : 480 * half + 480]
                    if half == 0:
                        nc.scalar.activation(dst, pt[:, :], mybir.ActivationFunctionType.Copy)
                    else:
                        nc.vector.tensor_copy(dst, pt[:, :])

                # --- MM2: horizontal conv 1 + 16->16 mix -> z2[(img,r2), (h30, w'30)]
                z2 = z2p.tile([128, PIX2], BF16)
                t1v = t1[:, :].rearrange("p (h w) -> p h w", w=32)
                for half in range(2):
                    pz = ps2.tile([128, 450], F32)
                    for dy in range(3):
                        nc.tensor.matmul(
                            pz[:, :],
                            w2[:, 128 * dy : 128 * dy + 128],
                            t1v[:, 15 * half : 15 * half + 15, dy : dy + 30],
                            start=(dy == 0),
                            stop=(dy == 2),
                        )
                    dst = z2[:, 450 * half : 450 * half + 450]
                    if half == 0:
                        nc.scalar.activation(dst, pz[:, :], mybir.ActivationFunctionType.Copy)
                    else:
                        nc.vector.tensor_copy(dst, pz[:, :])

                # --- transpose z2 into feature-major chunks (DMA xbar transpose)
                for c in range(NCHUNK):
                    off = 128 * c if c < NCHUNK - 1 else PIX2 - 128
                    nc.sync.dma_start(
                        tall[c][:, 128 * g : 128 * g + 128],
                        z2[:, off : off + 128],
                        transpose=True,
                    )

            # --- classifier: logits[img, n] over K = (chunk, r, 128 pix)
            psL = psl.tile([B_LOC, NC], F32)
            k = 0
            nmm = NCHUNK * R
            for c in range(NCHUNK):
                tv = tall[c][:, :].rearrange("p (i r) -> p r i", r=16)
                for r in range(R):
                    nc.tensor.matmul(
                        psL[:, :],
                        tv[:, r, :],
                        wc[:, NC * (R * c + r) : NC * (R * c + r) + NC],
                        start=(k == 0),
                        stop=(k == nmm - 1),
                    )
                    k += 1

            # --- bias + log_softmax on [64, 10]
            lt = smx.tile([B_LOC, NC], F32)
            nc.vector.tensor_add(lt[:, :], psL[:, :], bc[:, :])
            m = smx.tile([B_LOC, 1], F32)
            nc.vector.tensor_reduce(m[:, :], lt[:, :], axis=mybir.AxisListType.X,
                                    op=mybir.AluOpType.max)
            mneg = smx.tile([B_LOC, 1], F32)
            nc.vector.tensor_scalar_mul(mneg[:, :], m[:, :], -1.0)
            e = smx.tile([B_LOC, NC], F32)
            nc.scalar.activation(e[:, :], lt[:, :], mybir.ActivationFunctionType.Exp,
                                 bias=mneg[:, :], scale=1.0)
            s = smx.tile([B_LOC, 1], F32)
            nc.vector.tensor_reduce(s[:, :], e[:, :], axis=mybir.AxisListType.X,
                                    op=mybir.AluOpType.add)
            ls = smx.tile([B_LOC, 1], F32)
            nc.scalar.activation(ls[:, :], s[:, :], mybir.ActivationFunctionType.Ln)
            o = smx.tile([B_LOC, NC], F32)
            nc.vector.tensor_scalar(o[:, :], lt[:, :], mneg[:, :], ls[:, :],
                                    op0=mybir.AluOpType.add,
                                    op1=mybir.AluOpType.subtract)
            nc.sync.dma_start(out_d[:, :], o[:, :])

    nc.compile()
    return nc


def _host_weights(l1_f0, l1_f1, l1_f2, l1_f3, l2_f0, l2_f1, l2_f2, l2_f3, W_cls, b_cls):
    l1_f0, l1_f1, l1_f2, l1_f3 = (np.asarray(a, np.float32) for a in (l1_f0, l1_f1, l1_f2, l1_f3))
    l2_f0, l2_f1, l2_f2, l2_f3 = (np.asarray(a, np.float32) for a in (l2_f0, l2_f1, l2_f2, l2_f3))
    W_cls = np.asarray(W_cls, np.float32)
    b_cls = np.asarray(b_cls, np.float32)

    # MM1 weights: rows (dx, img, c), cols (img, r)
    w1 = np.zeros((72, 128), np.float32)
    for dx in range(3):
        blk = l1_f3 * l1_f1[dx][None, :]  # [c, r]
        for img in range(G):
            r0, c0 = 24 * dx + 3 * img, 16 * img
            w1[r0 : r0 + 3, c0 : c0 + 16] = blk

    # MM2 weights: rows (img, r), cols (dy; img, r2)
    M1 = l1_f0.T @ l2_f3  # [r, r2]
    w2 = np.zeros((128, 3 * 128), np.float32)
    for dy in range(3):
        H = l1_f2[dy][:, None] * M1  # [r, r2]
        for img in range(G):
            r0, c0 = 16 * img, 128 * dy + 16 * img
            w2[r0 : r0 + 16, c0 : c0 + 16] = H

    # Classifier folded weights: Wc3[n, r2, h, w] on the 30x30 z2 grid
    Wc2 = np.einsum("nfhw,fr->nrhw", W_cls.reshape(NC, 32, 28, 28), l2_f0)
    Wc3 = np.zeros((NC, R, H2, W2), np.float32)
    for dx in range(3):
        for dy in range(3):
            Wc3[:, :, dx : dx + 28, dy : dy + 28] += (
                Wc2 * (l2_f1[dx] * l2_f2[dy])[None, :, None, None]
            )
    Wc3f = Wc3.transpose(1, 2, 3, 0).reshape(R, PIX2, NC)  # [r, pix, n]
    wc = np.zeros((128, NCHUNK, R, NC), np.float32)
    for c in range(NCHUNK - 1):
        wc[:, c] = Wc3f[:, 128 * c : 128 * c + 128].transpose(1, 0, 2)
    # last chunk covers pix 772..899 but only 896..899 are not already counted
    wc[124:128, NCHUNK - 1] = Wc3f[:, 896:900].transpose(1, 0, 2)
    wc = wc.reshape(128, NCHUNK * R * NC).astype(ml_dtypes.bfloat16)

    bc = np.tile(b_cls[None, :], (B_LOC, 1)).astype(np.float32)
    w1 = w1.astype(ml_dtypes.bfloat16)
    w2 = w2.astype(ml_dtypes.bfloat16)
    return w1, w2, wc, bc


def _prepare_in_maps(x, l1_f0, l1_f1, l1_f2, l1_f3, l2_f0, l2_f1, l2_f2, l2_f3,
                     W_cls, b_cls):
    w1, w2, wc, bc = _host_weights(
        l1_f0, l1_f1, l1_f2, l1_f3, l2_f0, l2_f1, l2_f2, l2_f3, W_cls, b_cls
    )
    x = np.asarray(x, np.float32).reshape(B, 3, 1024).astype(ml_dtypes.bfloat16)
    in_maps = []
    for i in range(N_CORES):
        xs = np.ascontiguousarray(
            x[B_LOC * i : B_LOC * (i + 1)].reshape(B_LOC * 3, 1024)
        )
        in_maps.append({"x": xs, "w1": w1, "w2": w2, "wc": wc, "bc": bc})
    return in_maps


def kernel(x, l1_f0, l1_f1, l1_f2, l1_f3, l2_f0, l2_f1, l2_f2, l2_f3, W_cls, b_cls):
    if "nc" not in _CACHE:
        _CACHE["nc"] = _build_nc()
    nc = _CACHE["nc"]

    in_maps = _prepare_in_maps(x, l1_f0, l1_f1, l1_f2, l1_f3,
                               l2_f0, l2_f1, l2_f2, l2_f3, W_cls, b_cls)
    res = run_bass_kernel_spmd(nc, in_maps, list(range(N_CORES))).results
    out = np.concatenate([res[i]["out"] for i in range(N_CORES)], axis=0)
    return out.astype(np.float32)


# revision 13
# speedup vs baseline: 1.9263x; 1.9263x over previous
"""Trainium2 Bass kernel for the CP-decomposed 2-layer CNN + classifier.

Network (per image):
  x (3,32,32) -> CP conv1 -> h1 (32,30,30) -> CP conv2 -> h2 (32,28,28)
  -> flatten -> W_cls -> log_softmax  -> (10,)

The CP (rank-16) structure makes every layer separable. We fuse:
  MM1  = channel-contract (C3->R16) + vertical 3-tap conv of layer 1
         (3 h-shifted matmuls accumulated in PSUM, K = 8img*3c = 24)
  MM2  = horizontal conv of layer 1 + (layer-1 expand @ layer-2 contract)
         as a single 16->16 pointwise map, 3 w-shifted matmuls PSUM-accumulated
  CLS  = layer-2 vertical+horizontal convs + layer-2 expand + classifier,
         all folded into one (10 x 16*30*30) weight matrix precomputed on
         host, applied on PE-transposed (feature-major) activations with
         4-way column-tiled concurrent matmuls.

Data-parallel over batch: 512 images -> 8 cores x 64 images.
Per core, images are processed in 8 groups of 8 (partition dim = 8 img x 16 rank).
"""

import sys

sys.path.insert(0, "/opt/trn_rl_repo")

import numpy as np
import ml_dtypes

import concourse.bass as bass
import concourse.bacc as bacc
import concourse.mybir as mybir
import concourse.tile as tile
from concourse.bass_utils import run_bass_kernel_spmd
from concourse.masks import make_identity

F32 = mybir.dt.float32
BF16 = mybir.dt.bfloat16
AF = mybir.ActivationFunctionType

N_CORES = 8
B = 512
B_LOC = B // N_CORES  # 64 images per core
G = 8                 # images per group
NG = B_LOC // G       # 8 groups
R = 16                # CP rank
NC = 10               # classes

H1, W1 = 30, 32       # t1 grid (after vertical conv of layer 1)
H2, W2 = 30, 30       # z2 grid (after horizontal conv + channel mix)
PIX2 = H2 * W2        # 900
NCHUNK = 8            # transpose chunks of 128 pix (last overlaps, zero-masked)

_CACHE = {}


def _build_nc():
    nc = bacc.Bacc()
    x_d = nc.dram_tensor("x", [B_LOC * 3, 1024], BF16, kind="ExternalInput")
    w1_d = nc.dram_tensor("w1", [24, 3 * 128], BF16, kind="ExternalInput")
    w2_d = nc.dram_tensor("w2", [128, 3 * 128], BF16, kind="ExternalInput")
    wc_d = nc.dram_tensor("wc", [128, NCHUNK * R * NC], BF16, kind="ExternalInput")
    bc_d = nc.dram_tensor("bc", [NC, 1], F32, kind="ExternalInput")
    out_d = nc.dram_tensor("out", [B_LOC, NC], F32, kind="ExternalOutput")

    with tile.TileContext(nc) as tc:
        with (
            tc.tile_pool(name="wpool", bufs=1) as wp,
            tc.tile_pool(name="xp", bufs=3) as xp,
            tc.tile_pool(name="t1p", bufs=2) as t1p,
            tc.tile_pool(name="z2p", bufs=2) as z2p,
            tc.tile_pool(name="tallp", bufs=1) as tallp,
            tc.tile_pool(name="smx", bufs=1) as smx,
            tc.tile_pool(name="ps1", bufs=2, space="PSUM") as ps1,
            tc.tile_pool(name="ps2", bufs=2, space="PSUM") as ps2,
            tc.tile_pool(name="pst", bufs=2, space="PSUM") as pst,
            tc.tile_pool(name="psl", bufs=2, space="PSUM") as psl,
        ):
            w1 = wp.tile([24, 3 * 128], BF16)
            nc.gpsimd.dma_start(w1[:, :], w1_d[:, :])
            w2 = wp.tile([128, 3 * 128], BF16)
            nc.gpsimd.dma_start(w2[:, :], w2_d[:, :])
            wc = wp.tile([128, NCHUNK * R * NC], BF16)
            nc.gpsimd.dma_start(wc[:, :], wc_d[:, :])
            bc = wp.tile([NC, 1], F32)
            nc.gpsimd.dma_start(bc[:, :], bc_d[:, :])
            idb = wp.tile([128, 128], BF16)
            make_identity(nc, idb[:, :])
            idf = wp.tile([16, 16], F32)
            make_identity(nc, idf[:, :])

            tall = [
                tallp.tile([128, 1024], BF16, name=f"tall{c}", tag=f"tall{c}")
                for c in range(NCHUNK)
            ]

            for g in range(NG):
                # --- load x for this group (alternate HWDGE queues)
                xg = xp.tile([24, 1024], BF16)
                dma_eng = nc.sync if g % 2 == 0 else nc.scalar
                dma_eng.dma_start(xg[:, :], x_d[24 * g : 24 * g + 24, :])

                # --- MM1: channel contract + vertical conv 1 -> t1[(img,r), (h30,w32)]
                t1 = t1p.tile([128, H1 * W1], BF16)
                xv = xg[0:24, :].rearrange("p (h w) -> p h w", w=32)
                for half in range(2):
                    pt = ps1.tile([128, 480], F32)
                    for dx in range(3):
                        nc.tensor.matmul(
                            pt[:, :],
                            w1[:, 128 * dx : 128 * dx + 128],
                            xv[:, 15 * half + dx : 15 * half + dx + 15, :],
                            start=(dx == 0),
                            stop=(dx == 2),
                        )
                    dst = t1[:, 480 * half : 480 * half + 480]
                    if half == 0:
                        nc.scalar.activation(dst, pt[:, :], AF.Copy)
                    else:
                        nc.vector.tensor_copy(dst, pt[:, :])

                # --- MM2: horizontal conv 1 + 16->16 mix -> z2[(img,r2), (h30,w'30)]
                z2 = z2p.tile([128, PIX2], BF16)
                t1v = t1[:, :].rearrange("p (h w) -> p h w", w=32)
                for half in range(2):
                    pz = ps2.tile([128, 450], F32)
                    for dy in range(3):
                        nc.tensor.matmul(
                            pz[:, :],
                            w2[:, 128 * dy : 128 * dy + 128],
                            t1v[:, 15 * half : 15 * half + 15, dy : dy + 30],
                            start=(dy == 0),
                            stop=(dy == 2),
                        )
                    dst = z2[:, 450 * half : 450 * half + 450]
                    if half == 0:
                        nc.scalar.activation(dst, pz[:, :], AF.Copy)
                    else:
                        nc.vector.tensor_copy(dst, pz[:, :])

                # --- PE-transpose z2 into feature-major chunks
                for c in range(NCHUNK):
                    off = 128 * c if c < NCHUNK - 1 else PIX2 - 128
                    pT = pst.tile([128, 128], BF16)
                    nc.tensor.transpose(pT[:, :], z2[:, off : off + 128], idb[:, :])
                    dst = tall[c][:, 128 * g : 128 * g + 128]
                    if c % 2 == 0:
                        nc.scalar.activation(dst, pT[:, :], AF.Copy)
                    else:
                        nc.vector.tensor_copy(dst, pT[:, :])

            # --- classifier: 2 column-group-concurrent accumulation chains
            # chain j covers chunks 4j..4j+3, writes partitions 32j..32j+10
            # of its own PSUM bank (zero regions are bank-granular).
            psLs = [
                psl.tile([128, B_LOC], F32, name=f"clsps{j}", tag="clsps")
                for j in range(2)
            ]
            for c2 in range(4):
                for r in range(R):
                    for j in range(2):
                        c = 4 * j + c2
                        tv = tall[c][:, :].rearrange("p (i r) -> p r i", r=16)
                        nc.tensor.matmul(
                            psLs[j][32 * j : 32 * j + NC, :],
                            wc[:, NC * (R * c + r) : NC * (R * c + r) + NC],
                            tv[:, r, :],
                            start=(c2 == 0 and r == 0),
                            stop=(c2 == 3 and r == R - 1),
                            tile_position=(0, 32 * j),
                        )

            # --- combine the 2 chain partials + bias -> lt10 [10, 64]
            ltb = smx.tile([NC, B_LOC], F32)
            nc.scalar.activation(ltb[:, :], psLs[1][32 : 32 + NC, :], AF.Copy)
            lt10 = smx.tile([NC, B_LOC], F32)
            nc.vector.scalar_tensor_tensor(
                lt10[:, :], psLs[0][0:NC, :], bc[:, :], ltb[:, :],
                op0=mybir.AluOpType.add, op1=mybir.AluOpType.add,
            )

            # --- transpose to [64, 10] and log_softmax
            pT10 = psl.tile([B_LOC, NC], F32, name="pT10", tag="clsps")
            nc.tensor.transpose(pT10[:, :], lt10[:, :], idf[0:NC, 0:NC])
            lt = smx.tile([B_LOC, NC], F32)
            nc.vector.tensor_copy(lt[:, :], pT10[:, :])
            mneg = smx.tile([B_LOC, 1], F32)
            nc.vector.tensor_reduce(mneg[:, :], lt[:, :], axis=mybir.AxisListType.X,
                                    op=mybir.AluOpType.max, negate=True)
            e = smx.tile([B_LOC, NC], F32)
            nc.scalar.activation(e[:, :], lt[:, :], AF.Exp, bias=mneg[:, :], scale=1.0)
            s = smx.tile([B_LOC, 1], F32)
            nc.vector.tensor_reduce(s[:, :], e[:, :], axis=mybir.AxisListType.X,
                                    op=mybir.AluOpType.add)
            ls = smx.tile([B_LOC, 1], F32)
            nc.scalar.activation(ls[:, :], s[:, :], AF.Ln)
            o = smx.tile([B_LOC, NC], F32)
            nc.vector.tensor_scalar(o[:, :], lt[:, :], mneg[:, :], ls[:, :],
                                    op0=mybir.AluOpType.add,
                                    op1=mybir.AluOpType.subtract)
            nc.sync.dma_start(out_d[:, :], o[:, :])

    nc.compile()
    return nc


def _host_weights(l1_f0, l1_f1, l1_f2, l1_f3, l2_f0, l2_f1, l2_f2, l2_f3, W_cls, b_cls):
    l1_f0, l1_f1, l1_f2, l1_f3 = (np.asarray(a, np.float32) for a in (l1_f0, l1_f1, l1_f2, l1_f3))
    l2_f0, l2_f1, l2_f2, l2_f3 = (np.asarray(a, np.float32) for a in (l2_f0, l2_f1, l2_f2, l2_f3))
    W_cls = np.asarray(W_cls, np.float32)
    b_cls = np.asarray(b_cls, np.float32)

    # MM1 weights: [24 rows = (img,c), 3*128 cols = (dx; img,r)]
    w1 = np.zeros((24, 3, 128), np.float32)
    for dx in range(3):
        blk = l1_f3 * l1_f1[dx][None, :]  # [c, r]
        for img in range(G):
            w1[3 * img : 3 * img + 3, dx, 16 * img : 16 * img + 16] = blk
    w1 = w1.reshape(24, 3 * 128)

    # MM2 weights: rows (img, r), cols (dy; img, r2)
    M1 = l1_f0.T @ l2_f3  # [r, r2]
    w2 = np.zeros((128, 3 * 128), np.float32)
    for dy in range(3):
        H = l1_f2[dy][:, None] * M1  # [r, r2]
        for img in range(G):
            r0, c0 = 16 * img, 128 * dy + 16 * img
            w2[r0 : r0 + 16, c0 : c0 + 16] = H

    # Classifier folded weights: Wc3[n, r2, h, w] on the 30x30 z2 grid
    Wc2 = np.einsum("nfhw,fr->nrhw", W_cls.reshape(NC, 32, 28, 28), l2_f0)
    Wc3 = np.zeros((NC, R, H2, W2), np.float32)
    for dx in range(3):
        for dy in range(3):
            Wc3[:, :, dx : dx + 28, dy : dy + 28] += (
                Wc2 * (l2_f1[dx] * l2_f2[dy])[None, :, None, None]
            )
    Wc3f = Wc3.transpose(1, 2, 3, 0).reshape(R, PIX2, NC)  # [r, pix, n]
    wc = np.zeros((128, NCHUNK, R, NC), np.float32)
    for c in range(NCHUNK - 1):
        wc[:, c] = Wc3f[:, 128 * c : 128 * c + 128].transpose(1, 0, 2)
    # last chunk covers pix 772..899 but only 896..899 are not already counted
    wc[124:128, NCHUNK - 1] = Wc3f[:, 896:900].transpose(1, 0, 2)
    wc = wc.reshape(128, NCHUNK * R * NC).astype(ml_dtypes.bfloat16)

    bc = b_cls.reshape(NC, 1).astype(np.float32)
    w1 = w1.astype(ml_dtypes.bfloat16)
    w2 = w2.astype(ml_dtypes.bfloat16)
    return w1, w2, wc, bc


def _prepare_in_maps(x, l1_f0, l1_f1, l1_f2, l1_f3, l2_f0, l2_f1, l2_f2, l2_f3,
                     W_cls, b_cls):
    w1, w2, wc, bc = _host_weights(
        l1_f0, l1_f1, l1_f2, l1_f3, l2_f0, l2_f1, l2_f2, l2_f3, W_cls, b_cls
    )
    x = np.asarray(x, np.float32).reshape(B, 3, 1024).astype(ml_dtypes.bfloat16)
    in_maps = []
    for i in range(N_CORES):
        xs = np.ascontiguousarray(
            x[B_LOC * i : B_LOC * (i + 1)].reshape(B_LOC * 3, 1024)
        )
        in_maps.append({"x": xs, "w1": w1, "w2": w2, "wc": wc, "bc": bc})
    return in_maps


def kernel(x, l1_f0, l1_f1, l1_f2, l1_f3, l2_f0, l2_f1, l2_f2, l2_f3, W_cls, b_cls):
    if "nc" not in _CACHE:
        _CACHE["nc"] = _build_nc()
    nc = _CACHE["nc"]

    in_maps = _prepare_in_maps(x, l1_f0, l1_f1, l1_f2, l1_f3,
                               l2_f0, l2_f1, l2_f2, l2_f3, W_cls, b_cls)
    res = run_bass_kernel_spmd(nc, in_maps, list(range(N_CORES))).results
    out = np.concatenate([res[i]["out"] for i in range(N_CORES)], axis=0)
    return out.astype(np.float32)


# revision 15
# speedup vs baseline: 2.2317x; 1.1586x over previous
"""Trainium2 Bass kernel for the CP-decomposed 2-layer CNN + classifier.

Network (per image):
  x (3,32,32) -> CP conv1 -> h1 (32,30,30) -> CP conv2 -> h2 (32,28,28)
  -> flatten -> W_cls -> log_softmax  -> (10,)

The CP (rank-16) structure makes every layer separable. We fuse:
  MM1  = channel-contract (C3->R16) + vertical 3-tap conv of layer 1
         (3 h-shifted matmuls accumulated in PSUM, K = 8img*3c = 24)
  MM2  = horizontal conv of layer 1 + (layer-1 expand @ layer-2 contract)
         as a single 16->16 pointwise map, 3 w-shifted matmuls PSUM-accumulated
  CLS  = layer-2 vertical+horizontal convs + layer-2 expand + classifier,
         all folded into one (10 x 16*30*30) weight matrix precomputed on
         host, applied on PE-transposed (feature-major) activations with
         4-way column-tiled concurrent matmuls.

Data-parallel over batch: 512 images -> 8 cores x 64 images.
Per core, images are processed in 8 groups of 8 (partition dim = 8 img x 16 rank).
"""

import sys

sys.path.insert(0, "/opt/trn_rl_repo")

import numpy as np
import ml_dtypes

import concourse.bass as bass
import concourse.bacc as bacc
import concourse.mybir as mybir
import concourse.tile as tile
from concourse.bass_utils import run_bass_kernel_spmd
from concourse.masks import make_identity

F32 = mybir.dt.float32
BF16 = mybir.dt.bfloat16
AF = mybir.ActivationFunctionType

N_CORES = 8
B = 512
B_LOC = B // N_CORES  # 64 images per core
G = 8                 # images per group
NG = B_LOC // G       # 8 groups
R = 16                # CP rank
NC = 10               # classes

H1, W1 = 30, 32       # t1 grid (after vertical conv of layer 1)
H2, W2 = 30, 30       # z2 grid (after horizontal conv + channel mix)
PIX2 = H2 * W2        # 900
NCHUNK = 8            # transpose chunks of 128 pix (last overlaps, zero-masked)

_CACHE = {}


def _build_nc():
    nc = bacc.Bacc()
    x_d = nc.dram_tensor("x", [B_LOC * 3, 1024], BF16, kind="ExternalInput")
    w1_d = nc.dram_tensor("w1", [24, 3 * 128], BF16, kind="ExternalInput")
    w2_d = nc.dram_tensor("w2", [128, 3 * 128], BF16, kind="ExternalInput")
    wc_d = nc.dram_tensor("wc", [128, NCHUNK * R * NC], BF16, kind="ExternalInput")
    bc_d = nc.dram_tensor("bc", [NC, 1], F32, kind="ExternalInput")
    out_d = nc.dram_tensor("out", [B_LOC, NC], F32, kind="ExternalOutput")

    with tile.TileContext(nc) as tc:
        with (
            tc.tile_pool(name="wpool", bufs=1) as wp,
            tc.tile_pool(name="xp", bufs=3) as xp,
            tc.tile_pool(name="t1p", bufs=2) as t1p,
            tc.tile_pool(name="z2p", bufs=2) as z2p,
            tc.tile_pool(name="tallp", bufs=1) as tallp,
            tc.tile_pool(name="smx", bufs=1) as smx,
            tc.tile_pool(name="ps1", bufs=2, space="PSUM") as ps1,
            tc.tile_pool(name="ps2", bufs=2, space="PSUM") as ps2,
            tc.tile_pool(name="pst", bufs=2, space="PSUM") as pst,
            tc.tile_pool(name="psl", bufs=2, space="PSUM") as psl,
        ):
            w1 = wp.tile([24, 3 * 128], BF16)
            nc.gpsimd.dma_start(w1[:, :], w1_d[:, :])
            w2 = wp.tile([128, 3 * 128], BF16)
            nc.gpsimd.dma_start(w2[:, :], w2_d[:, :])
            wc = wp.tile([128, NCHUNK * R * NC], BF16)
            nc.gpsimd.dma_start(wc[:, :], wc_d[:, :])
            bc = wp.tile([NC, 1], F32)
            nc.gpsimd.dma_start(bc[:, :], bc_d[:, :])
            idb = wp.tile([128, 128], BF16)
            make_identity(nc, idb[:, :])
            idf = wp.tile([16, 16], F32)
            make_identity(nc, idf[:, :])

            TALL = tallp.tile([128, NCHUNK * 1024], BF16, name="TALL")

            def emit_transposes(g, z2):
                pT = pst.tile([128, 1024], BF16, name="pT", tag="pT")
                for c in range(NCHUNK):
                    off = 128 * c if c < NCHUNK - 1 else PIX2 - 128
                    nc.tensor.transpose(
                        pT[:, 128 * c : 128 * c + 128],
                        z2[:, off : off + 128],
                        idb[:, :],
                    )
                dst3 = TALL[:, :].rearrange("p (c gi) -> p c gi", gi=1024)[
                    :, :, 128 * g : 128 * g + 128
                ]
                src3 = pT[:, :].rearrange("p (c i) -> p c i", i=128)
                if g % 2 == 0:
                    nc.scalar.activation(dst3, src3, AF.Copy)
                else:
                    nc.vector.tensor_copy(dst3, src3)

            pending = None
            for g in range(NG):
                # --- load x for this group (alternate DMA queues)
                xg = xp.tile([24, 1024], BF16)
                dma_eng = nc.sync if g % 2 == 0 else nc.gpsimd
                dma_eng.dma_start(xg[:, :], x_d[24 * g : 24 * g + 24, :])

                # --- MM1: channel contract + vertical conv 1 -> t1[(img,r), (h30,w32)]
                t1 = t1p.tile([128, H1 * W1], BF16)
                xv = xg[0:24, :].rearrange("p (h w) -> p h w", w=32)
                for half in range(2):
                    pt = ps1.tile([128, 480], F32)
                    for dx in range(3):
                        nc.tensor.matmul(
                            pt[:, :],
                            w1[:, 128 * dx : 128 * dx + 128],
                            xv[:, 15 * half + dx : 15 * half + dx + 15, :],
                            start=(dx == 0),
                            stop=(dx == 2),
                        )
                    dst = t1[:, 480 * half : 480 * half + 480]
                    nc.scalar.activation(dst, pt[:, :], AF.Copy)

                # --- MM2: horizontal conv 1 + 16->16 mix -> z2[(img,r2), (h30,w'30)]
                z2 = z2p.tile([128, PIX2], BF16)
                t1v = t1[:, :].rearrange("p (h w) -> p h w", w=32)
                for half in range(2):
                    pz = ps2.tile([128, 450], F32)
                    for dy in range(3):
                        nc.tensor.matmul(
                            pz[:, :],
                            w2[:, 128 * dy : 128 * dy + 128],
                            t1v[:, 15 * half : 15 * half + 15, dy : dy + 30],
                            start=(dy == 0),
                            stop=(dy == 2),
                        )
                    dst = z2[:, 450 * half : 450 * half + 450]
                    nc.vector.tensor_copy(dst, pz[:, :])

                # --- PE-transpose previous group (skewed so PE never waits)
                if pending is not None:
                    emit_transposes(*pending)
                pending = (g, z2)
            emit_transposes(*pending)

            # --- classifier: 2 column-group-concurrent accumulation chains
            # chain j covers chunks 4j..4j+3, writes partitions 32j..32j+10
            # of its own PSUM bank (zero regions are bank-granular).
            psLs = [
                psl.tile([128, B_LOC], F32, name=f"clsps{j}", tag="clsps")
                for j in range(2)
            ]
            for c2 in range(4):
                for r in range(R):
                    for j in range(2):
                        c = 4 * j + c2
                        tv = TALL[:, 1024 * c : 1024 * (c + 1)].rearrange("p (i r) -> p r i", r=16)
                        nc.tensor.matmul(
                            psLs[j][32 * j : 32 * j + NC, :],
                            wc[:, NC * (R * c + r) : NC * (R * c + r) + NC],
                            tv[:, r, :],
                            start=(c2 == 0 and r == 0),
                            stop=(c2 == 3 and r == R - 1),
                            tile_position=(0, 32 * j),
                        )

            # --- combine the 2 chain partials + bias -> lt10 [10, 64]
            ltb = smx.tile([NC, B_LOC], F32)
            nc.scalar.activation(ltb[:, :], psLs[1][32 : 32 + NC, :], AF.Copy)
            lt10 = smx.tile([NC, B_LOC], F32)
            nc.vector.scalar_tensor_tensor(
                lt10[:, :], psLs[0][0:NC, :], bc[:, :], ltb[:, :],
                op0=mybir.AluOpType.add, op1=mybir.AluOpType.add,
            )

            # --- transpose to [64, 10] and log_softmax
            pT10 = psl.tile([B_LOC, NC], F32, name="pT10", tag="clsps")
            nc.tensor.transpose(pT10[:, :], lt10[:, :], idf[0:NC, 0:NC])
            lt = smx.tile([B_LOC, NC], F32)
            nc.vector.tensor_copy(lt[:, :], pT10[:, :])
            mneg = smx.tile([B_LOC, 1], F32)
            nc.vector.tensor_reduce(mneg[:, :], lt[:, :], axis=mybir.AxisListType.X,
                                    op=mybir.AluOpType.max, negate=True)
            e = smx.tile([B_LOC, NC], F32)
            nc.scalar.activation(e[:, :], lt[:, :], AF.Exp, bias=mneg[:, :], scale=1.0)
            s = smx.tile([B_LOC, 1], F32)
            nc.vector.tensor_reduce(s[:, :], e[:, :], axis=mybir.AxisListType.X,
                                    op=mybir.AluOpType.add)
            ls = smx.tile([B_LOC, 1], F32)
            nc.scalar.activation(ls[:, :], s[:, :], AF.Ln)
            o = smx.tile([B_LOC, NC], F32)
            nc.vector.tensor_scalar(o[:, :], lt[:, :], mneg[:, :], ls[:, :],
                                    op0=mybir.AluOpType.add,
                                    op1=mybir.AluOpType.subtract)
            nc.sync.dma_start(out_d[:, :], o[:, :])

    nc.compile()
    return nc


def _host_weights(l1_f0, l1_f1, l1_f2, l1_f3, l2_f0, l2_f1, l2_f2, l2_f3, W_cls, b_cls):
    l1_f0, l1_f1, l1_f2, l1_f3 = (np.asarray(a, np.float32) for a in (l1_f0, l1_f1, l1_f2, l1_f3))
    l2_f0, l2_f1, l2_f2, l2_f3 = (np.asarray(a, np.float32) for a in (l2_f0, l2_f1, l2_f2, l2_f3))
    W_cls = np.asarray(W_cls, np.float32)
    b_cls = np.asarray(b_cls, np.float32)

    # MM1 weights: [24 rows = (img,c), 3*128 cols = (dx; img,r)]
    w1 = np.zeros((24, 3, 128), np.float32)
    for dx in range(3):
        blk = l1_f3 * l1_f1[dx][None, :]  # [c, r]
        for img in range(G):
            w1[3 * img : 3 * img + 3, dx, 16 * img : 16 * img + 16] = blk
    w1 = w1.reshape(24, 3 * 128)

    # MM2 weights: rows (img, r), cols (dy; img, r2)
    M1 = l1_f0.T @ l2_f3  # [r, r2]
    w2 = np.zeros((128, 3 * 128), np.float32)
    for dy in range(3):
        H = l1_f2[dy][:, None] * M1  # [r, r2]
        for img in range(G):
            r0, c0 = 16 * img, 128 * dy + 16 * img
            w2[r0 : r0 + 16, c0 : c0 + 16] = H

    # Classifier folded weights: Wc3[n, r2, h, w] on the 30x30 z2 grid
    Wc2 = np.einsum("nfhw,fr->nrhw", W_cls.reshape(NC, 32, 28, 28), l2_f0)
    Wc3 = np.zeros((NC, R, H2, W2), np.float32)
    for dx in range(3):
        for dy in range(3):
            Wc3[:, :, dx : dx + 28, dy : dy + 28] += (
                Wc2 * (l2_f1[dx] * l2_f2[dy])[None, :, None, None]
            )
    Wc3f = Wc3.transpose(1, 2, 3, 0).reshape(R, PIX2, NC)  # [r, pix, n]
    wc = np.zeros((128, NCHUNK, R, NC), np.float32)
    for c in range(NCHUNK - 1):
        wc[:, c] = Wc3f[:, 128 * c : 128 * c + 128].transpose(1, 0, 2)
    # last chunk covers pix 772..899 but only 896..899 are not already counted
    wc[124:128, NCHUNK - 1] = Wc3f[:, 896:900].transpose(1, 0, 2)
    wc = wc.reshape(128, NCHUNK * R * NC).astype(ml_dtypes.bfloat16)

    bc = b_cls.reshape(NC, 1).astype(np.float32)
    w1 = w1.astype(ml_dtypes.bfloat16)
    w2 = w2.astype(ml_dtypes.bfloat16)
    return w1, w2, wc, bc


def _prepare_in_maps(x, l1_f0, l1_f1, l1_f2, l1_f3, l2_f0, l2_f1, l2_f2, l2_f3,
                     W_cls, b_cls):
    w1, w2, wc, bc = _host_weights(
        l1_f0, l1_f1, l1_f2, l1_f3, l2_f0, l2_f1, l2_f2, l2_f3, W_cls, b_cls
    )
    x = np.asarray(x, np.float32).reshape(B, 3, 1024).astype(ml_dtypes.bfloat16)
    in_maps = []
    for i in range(N_CORES):
        xs = np.ascontiguousarray(
            x[B_LOC * i : B_LOC * (i + 1)].reshape(B_LOC * 3, 1024)
        )
        in_maps.append({"x": xs, "w1": w1, "w2": w2, "wc": wc, "bc": bc})
    return in_maps


def kernel(x, l1_f0, l1_f1, l1_f2, l1_f3, l2_f0, l2_f1, l2_f2, l2_f3, W_cls, b_cls):
    if "nc" not in _CACHE:
        _CACHE["nc"] = _build_nc()
    nc = _CACHE["nc"]

    in_maps = _prepare_in_maps(x, l1_f0, l1_f1, l1_f2, l1_f3,
                               l2_f0, l2_f1, l2_f2, l2_f3, W_cls, b_cls)
    res = run_bass_kernel_spmd(nc, in_maps, list(range(N_CORES))).results
    out = np.concatenate([res[i]["out"] for i in range(N_CORES)], axis=0)
    return out.astype(np.float32)


# revision 16
# speedup vs baseline: 7.4528x; 3.3395x over previous
"""Trainium2 Bass kernel for the CP-decomposed 2-layer CNN + classifier.

Key observation: the reference network (two CP-factored convs + linear
classifier) is LINEAR up to the final log_softmax. The whole model
therefore folds, on the host, into a single affine map
    logits = A @ x_flat + b         A: (10, 3*32*32)
A is computed exactly from the CP factors by pushing the classifier
weights backward through both (separable) conv layers — O(10*16*1024)
host work, independent of batch size.

The device kernel is then just: logits = xT.T @ A.T per 128-row feature
chunk (24 chunks, PSUM-accumulated, 2 column-group-concurrent chains)
followed by a fused log_softmax. x is laid out feature-major on the host
so no on-device transposes are needed.

Data-parallel over batch: 512 images -> 8 cores x 64 images.
"""

import sys

sys.path.insert(0, "/opt/trn_rl_repo")

import numpy as np
import ml_dtypes

import concourse.bacc as bacc
import concourse.mybir as mybir
import concourse.tile as tile
from concourse.bass_utils import run_bass_kernel_spmd

F32 = mybir.dt.float32
BF16 = mybir.dt.bfloat16
AF = mybir.ActivationFunctionType

N_CORES = 8
B = 512
B_LOC = B // N_CORES   # 64 images per core
NC = 10                # classes
KF = 3 * 32 * 32       # 3072 input features
NCHUNK = KF // 128     # 24 feature chunks

_CACHE = {}


def _build_nc():
    nc = bacc.Bacc()
    # x, feature-major: xt[p, 64*c + i] = x_flat[img i, 128*c + p]
    xt_d = nc.dram_tensor("xt", [128, NCHUNK * B_LOC], BF16, kind="ExternalInput")
    # A chunks: a[p, 10*c + n] = A[n, 128*c + p]
    a_d = nc.dram_tensor("a", [128, NCHUNK * NC], BF16, kind="ExternalInput")
    bc_d = nc.dram_tensor("bc", [B_LOC, NC], F32, kind="ExternalInput")
    out_d = nc.dram_tensor("out", [B_LOC, NC], F32, kind="ExternalOutput")

    H = NCHUNK // 2  # chunks per chain

    with tile.TileContext(nc) as tc:
        with (
            tc.tile_pool(name="wp", bufs=1) as wp,
            tc.tile_pool(name="smx", bufs=1) as smx,
            tc.tile_pool(name="ps", bufs=2, space="PSUM") as ps,
        ):
            xt = wp.tile([128, NCHUNK * B_LOC], BF16)
            # split the big load across the three DMA queues
            third = NCHUNK // 3  # 8 chunks each
            for q, eng in enumerate((nc.sync, nc.scalar, nc.gpsimd)):
                lo = q * third * B_LOC
                hi = (q + 1) * third * B_LOC
                eng.dma_start(xt[:, lo:hi], xt_d[:, lo:hi])
            asb = wp.tile([128, NCHUNK * NC], BF16)
            nc.sync.dma_start(asb[:, :], a_d[:, :])
            bc = wp.tile([B_LOC, NC], F32)
            nc.scalar.dma_start(bc[:, :], bc_d[:, :])

            # two concurrent accumulation chains over feature chunks
            psA = ps.tile([128, NC], F32, name="psA", tag="cls")
            psB = ps.tile([128, NC], F32, name="psB", tag="cls")
            for s in range(H):
                for j in range(2):
                    c = H * j + s
                    out_ap = psA[0:B_LOC, :] if j == 0 else psB[64 : 64 + B_LOC, :]
                    nc.tensor.matmul(
                        out_ap,
                        xt[:, B_LOC * c : B_LOC * (c + 1)],
                        asb[:, NC * c : NC * (c + 1)],
                        start=(s == 0),
                        stop=(s == H - 1),
                        tile_position=(0, 64 * j),
                    )

            # combine chains + bias -> lt [64, 10] fp32
            ltb = smx.tile([B_LOC, NC], F32)
            nc.scalar.activation(ltb[:, :], psB[64 : 64 + B_LOC, :], AF.Copy)
            tmp = smx.tile([B_LOC, NC], F32)
            nc.vector.tensor_add(tmp[:, :], psA[0:B_LOC, :], ltb[:, :])
            lt = smx.tile([B_LOC, NC], F32)
            nc.vector.tensor_add(lt[:, :], tmp[:, :], bc[:, :])

            # log_softmax
            mneg = smx.tile([B_LOC, 1], F32)
            nc.vector.tensor_reduce(mneg[:, :], lt[:, :], axis=mybir.AxisListType.X,
                                    op=mybir.AluOpType.max, negate=True)
            e = smx.tile([B_LOC, NC], F32)
            nc.scalar.activation(e[:, :], lt[:, :], AF.Exp, bias=mneg[:, :], scale=1.0)
            s_ = smx.tile([B_LOC, 1], F32)
            nc.vector.tensor_reduce(s_[:, :], e[:, :], axis=mybir.AxisListType.X,
                                    op=mybir.AluOpType.add)
            ls = smx.tile([B_LOC, 1], F32)
            nc.scalar.activation(ls[:, :], s_[:, :], AF.Ln)
            o = smx.tile([B_LOC, NC], F32)
            nc.vector.tensor_scalar(o[:, :], lt[:, :], mneg[:, :], ls[:, :],
                                    op0=mybir.AluOpType.add,
                                    op1=mybir.AluOpType.subtract)
            nc.sync.dma_start(out_d[:, :], o[:, :])

    nc.compile()
    return nc


def _fold_affine(l1_f0, l1_f1, l1_f2, l1_f3, l2_f0, l2_f1, l2_f2, l2_f3, W_cls, b_cls):
    """Fold the whole (linear) network into logits = A @ x_flat + b."""
    f = np.float64
    l1_f0, l1_f1, l1_f2, l1_f3 = (np.asarray(x, f) for x in (l1_f0, l1_f1, l1_f2, l1_f3))
    l2_f0, l2_f1, l2_f2, l2_f3 = (np.asarray(x, f) for x in (l2_f0, l2_f1, l2_f2, l2_f3))
    W_cls = np.asarray(W_cls, f)

    # classifier pulled through layer-2 expand: Wc2[n, r2, 28, 28]
    Wc2 = np.einsum("nfhw,fr->nrhw", W_cls.reshape(NC, 32, 28, 28), l2_f0)
    # ... through layer-2 spatial convs: Wc3[n, r2, 30, 30]
    Wc3 = np.zeros((NC, 16, 30, 30), f)
    for dx in range(3):
        for dy in range(3):
            Wc3[:, :, dx : dx + 28, dy : dy + 28] += (
                Wc2 * (l2_f1[dx] * l2_f2[dy])[None, :, None, None]
            )
    # ... through (layer-1 expand @ layer-2 channel contract) and layer-1
    # horizontal conv: WT[n, r, 30, 32]
    M1 = l1_f0.T @ l2_f3  # [r, r2]
    WT = np.zeros((NC, 16, 30, 32), f)
    for dy in range(3):
        Hdy = l1_f2[dy][:, None] * M1  # [r, r2]
        WT[:, :, :, dy : dy + 30] += np.einsum("nshw,rs->nrhw", Wc3, Hdy)
    # ... through layer-1 vertical conv and channel contract: A[n, c, 32, 32]
    A = np.zeros((NC, 3, 32, 32), f)
    for dx in range(3):
        Gdx = l1_f3 * l1_f1[dx][None, :]  # [c, r]
        A[:, :, dx : dx + 30, :] += np.einsum("nrhw,cr->nchw", WT, Gdx)
    return A.reshape(NC, KF), np.asarray(b_cls, f)


def _prepare_in_maps(x, l1_f0, l1_f1, l1_f2, l1_f3, l2_f0, l2_f1, l2_f2, l2_f3,
                     W_cls, b_cls):
    A, b = _fold_affine(l1_f0, l1_f1, l1_f2, l1_f3,
                        l2_f0, l2_f1, l2_f2, l2_f3, W_cls, b_cls)
    a_arr = np.ascontiguousarray(
        A.T.reshape(NCHUNK, 128, NC).transpose(1, 0, 2).reshape(128, NCHUNK * NC)
    ).astype(ml_dtypes.bfloat16)
    bc = np.tile(np.asarray(b, np.float32)[None, :], (B_LOC, 1)).astype(np.float32)

    x = np.asarray(x, np.float32).reshape(B, KF)
    in_maps = []
    for i in range(N_CORES):
        xs = x[B_LOC * i : B_LOC * (i + 1)]  # [64, 3072]
        xt = np.ascontiguousarray(
            xs.T.reshape(NCHUNK, 128, B_LOC).transpose(1, 0, 2).reshape(128, NCHUNK * B_LOC)
        ).astype(ml_dtypes.bfloat16)
        in_maps.append({"xt": xt, "a": a_arr, "bc": bc})
    return in_maps


def kernel(x, l1_f0, l1_f1, l1_f2, l1_f3, l2_f0, l2_f1, l2_f2, l2_f3, W_cls, b_cls):
    if "nc" not in _CACHE:
        _CACHE["nc"] = _build_nc()
    nc = _CACHE["nc"]

    in_maps = _prepare_in_maps(x, l1_f0, l1_f1, l1_f2, l1_f3,
                               l2_f0, l2_f1, l2_f2, l2_f3, W_cls, b_cls)
    res = run_bass_kernel_spmd(nc, in_maps, list(range(N_CORES))).results
    out = np.concatenate([res[i]["out"] for i in range(N_CORES)], axis=0)
    return out.astype(np.float32)
